# revision 5
# baseline (speedup 1.0000x reference)
"""Trainium2 Bass kernel for nn_MultiHeadAttModel_allin.

Data-parallel over batch across 8 cores (4 batches/core); per core the
work runs as two 128-row passes (2 batches x 64 agents), software-
pipelined so PE / ACT / DVE / GPSIMD stay concurrently busy:

  - gather: block-diagonal 2-timestep matmuls put neighbor actions on
    128 partitions (t-parity, batch-pair, padded action-dim)
  - enc: per-(E-half, row-group) N=512 matmuls into 2-bank psum tiles
    (separate psum banks per PE row group -- two row groups sharing a
    bank races on hardware); gelu evictions write a PARITY-MAJOR t axis
    so every ACT store is contiguous (softmax/sum over t are order-
    invariant, so only the host-side pe table is reordered)
  - pe add: DVE tensor_tensor against a 16-wide-replicated pe table
    (inner stride-1 runs of 16; 2-wide runs are ~3x slower on HW)
  - kv: K and V matmuls (N=256) share per-(c,t) stationary loads;
    3-deep psum rotation; K evicted [row,t,j] split ACT/DVE, V evicted
    [row,t,j] on ACT (transposed V eviction measured ~3.5x slower)
  - attention: merged 8-head f16 ops; qh broadcast multiply + d-tree on
    DVE; exp/softmax-normalization deferred to a final 1/sum scale
  - av: exp broadcast over d materialized via a 3-way GPSIMD/ACT/DVE
    split into the dead kh slot, then 2x-mode DVE multiply and t-tree
  - emission order hand-interleaves enc(bb1)/kv(bb0) and kv1/att0/av0
    so each engine's FIFO always has ready work (gelu+relu share an ACT
    table set, so interleaving them costs no table reloads)

Measured (repeat-loop slope 33->97, this container): ~169 us vs the
staged baseline's ~264 us under the identical methodology.
"""

import contextlib
import math
from functools import lru_cache

import numpy as np

B, N, E = 32, 64, 256
DV, NV, HIS, DK = 32, 8, 50, 9
NC = 8
BPC = B // NC
NBB = BPC // 2
NPR = HIS // 2
SQD = 3.0
NH_GP = 2           # heads on GPSIMD in split att/av blocks
JGP = (NV - NH_GP) * DV  # j index where the GP head range starts


def _positional_encoding():
    position = np.arange(HIS, dtype=np.float64)[:, None]
    div_term = np.exp(
        np.arange(0, E, 2, dtype=np.float64) * (-math.log(10000.0) / E)
    )
    pe = np.zeros((HIS, E), np.float64)
    pe[:, 0::2] = np.sin(position * div_term)
    pe[:, 1::2] = np.cos(position * div_term)
    return pe.astype(np.float32)


@lru_cache(maxsize=4)
def _build(repeat=0):
    import concourse.tile as tile
    from concourse import bacc, mybir
    from concourse.masks import make_identity

    f32 = mybir.dt.float32
    f16 = mybir.dt.float16
    AF = mybir.ActivationFunctionType
    OP = mybir.AluOpType
    AX = mybir.AxisListType

    nc = bacc.Bacc("TRN2", target_bir_lowering=False, debug=False)

    qfm_d = nc.dram_tensor("q_fm", [128, 2, 2, 128], f16, kind="ExternalInput").ap()
    kg2_d = nc.dram_tensor("kg2", [128, NBB, NPR, 128], f16, kind="ExternalInput").ap()
    adj2_d = nc.dram_tensor("adj2", [128, NPR, 64], f16, kind="ExternalInput").ap()
    wq_d = nc.dram_tensor("Wq16", [128, 2, E], f16, kind="ExternalInput").ap()
    wkv_d = nc.dram_tensor("Wkv16", [128, 2, 2 * E], f16, kind="ExternalInput").ap()
    wo_d = nc.dram_tensor("Wo16", [128, 2, E], f16, kind="ExternalInput").ap()
    we_d = nc.dram_tensor("We_rep4", [128, 2, 128], f16, kind="ExternalInput").ap()
    pex_d = nc.dram_tensor("pe_x2", [128, 2, HIS, 16], f16, kind="ExternalInput").ap()
    out_d = nc.dram_tensor("out", [BPC, N, E], f32, kind="ExternalOutput").ap()

    with tile.TileContext(nc) as tc:
        with (
            tc.tile_pool(name="const", bufs=1) as const,
            tc.tile_pool(name="work", bufs=1) as work,
            tc.tile_pool(name="xb", bufs=1) as xb,
            tc.tile_pool(name="small", bufs=2) as small,
            tc.tile_pool(name="P1", bufs=2, space="PSUM") as P1,
            tc.tile_pool(name="P2", bufs=2, space="PSUM") as P2,
            tc.For_i(0, repeat) if repeat else contextlib.nullcontext(),
        ):
            ident = const.tile([128, 128], f32, tag="ident", name="ident")
            make_identity(nc, ident)

            kg2 = const.tile([128, NBB, NPR, 128], f16, tag="kg2", name="kg2")
            adj2 = const.tile([128, NPR, 64], f16, tag="adj2", name="adj2")
            We4 = const.tile([128, 2, 128], f16, tag="We4", name="We4")
            q_fm = const.tile([128, 2, 2, 128], f16, tag="q_fm", name="q_fm")
            Wq16 = const.tile([128, 2, E], f16, tag="Wq16", name="Wq16")
            Wkv16 = const.tile([128, 2, 2 * E], f16, tag="Wkv16", name="Wkv16")
            Wo16 = const.tile([128, 2, E], f16, tag="Wo16", name="Wo16")
            pe_full = const.tile([128, 2, HIS, 16], f16, tag="pe_full",
                                 name="pe_full")
            nc.sync.dma_start(adj2, adj2_d)
            nc.sync.dma_start(kg2[:, 0], kg2_d[:, 0])
            nc.sync.dma_start(We4, we_d)
            nc.sync.dma_start(kg2[:, 1], kg2_d[:, 1])
            nc.sync.dma_start(pe_full, pex_d)
            nc.sync.dma_start(q_fm, qfm_d)
            nc.sync.dma_start(Wq16, wq_d)
            nc.sync.dma_start(Wkv16, wkv_d)
            nc.sync.dma_start(Wo16, wo_d)

            # ---------------- state shared across emission helpers ------
            qh16 = const.tile([128, 2, E], f16, tag="qh16", name="qh16")
            neigh2s, g2s, g2ws = [], [], []
            khs, vh2s, att16s, exp16s, rinvs, avouts = {}, {}, {}, {}, {}, {}

            # ---------------- qh ----------------
            for rt in range(2):
                psq = P2.tile([128, 512], f32, tag="p2", name="psq")
                for c in range(2):
                    nc.tensor.matmul(
                        psq[:, :E], q_fm[:, c, rt, :], Wq16[:, c, :],
                        start=(c == 0), stop=(c == 1),
                    )
                nc.scalar.activation(qh16[:, rt, :], psq[:, :E], AF.Relu)

            # ---------------- gathers (DVE evicts) ----------------
            for bb in range(NBB):
                neigh2 = work.tile([128, NPR, 64], f16, tag="neigh2",
                                   name=f"neigh2_{bb}", bufs=2)
                for g0 in range(0, NPR, 13):
                    gw = min(13, NPR - g0)
                    psg = P1.tile([128, 13, 64], f32, tag="p1", name="psg")
                    for j in range(gw):
                        nc.tensor.matmul(
                            psg[:, j, :], kg2[:, bb, g0 + j, :],
                            adj2[:, g0 + j, :], start=True, stop=True,
                        )
                    nc.scalar.copy(neigh2[:, g0:g0 + gw, :],
                                   psg[:, :gw, :])
                neigh2s.append(neigh2)

            # ---------------- emission helpers ----------------
            def alloc_g2(bb):
                g2 = work.tile([128, 2, HIS, 2, 64], f16, tag="g2",
                               name=f"g2_{bb}", bufs=2)
                g2s.append(g2)
                g2ws.append(
                    g2.rearrange("p c t b (m r) -> p c t (b m) r", r=16)
                )

            def enc_chunk(bb, c, q, p0):
                # one (c, q=(tp,b2)) block, timestep-pairs [p0, p0+pw):
                # two N<=512 matmuls (row group q) into a 2-bank psum tile,
                # one FD<=1024 gelu evict. g2 t-axis is PARITY-MAJOR:
                # t' = tp*NPR + pr, so the evict is contiguous.
                g2 = g2s[bb]
                pw = min(16, NPR - p0)
                tp, b2 = q >> 1, q & 1
                pse = P2.tile([128, 2, 512], f32, tag="p2", name="pse")
                for h in range(2):
                    hw_ = min(8, pw - 8 * h)
                    if hw_ <= 0:
                        break
                    nc.tensor.matmul(
                        pse[:, h, :hw_ * 64],
                        We4[32 * q:32 * q + 32, c, :],
                        neigh2s[bb][32 * q:32 * q + 32,
                                    p0 + 8 * h:p0 + 8 * h + hw_, :]
                        .rearrange("p a n -> p (a n)"),
                        start=True, stop=True, tile_position=(32 * q, 0),
                    )
                t0 = tp * NPR + p0
                nc.scalar.activation(
                    g2[:, c, t0:t0 + pw, b2, :],
                    pse.rearrange("p h (a n) -> p (h a) n", n=64)[:, :pw, :],
                    AF.Gelu,
                )

            def peadd(bb, c, lo, hi, eng):
                g2w = g2ws[bb]
                tt = eng.tensor_tensor
                tt(
                    g2w[:, c, lo:hi, :, :], g2w[:, c, lo:hi, :, :],
                    pe_full[:, c, lo:hi, None, :].to_broadcast(
                        (128, hi - lo, 8, 16)
                    ),
                    OP.add,
                )

            def alloc_kv(bb):
                khs[bb] = work.tile([128, HIS, E], f16, tag="kh",
                                    name=f"kh{bb}", bufs=2)
                vh2s[bb] = work.tile([128, HIS, E], f16, tag="vh2",
                                     name=f"vh{bb}", bufs=2)

            def k_group(bb, t0, kev_dve):
                g2 = g2s[bb]
                tg = min(4, HIS - t0)
                psK = P1.tile([128, 4, E], f32, tag="p1", name="psK")
                for tt in range(tg):
                    for c in range(2):
                        nc.tensor.matmul(
                            psK[:, tt, :],
                            g2[:, c, t0 + tt, :, :].rearrange("p a b -> p (a b)"),
                            Wkv16[:, c, 0:E],
                            start=(c == 0), stop=(c == 1),
                        )
                if kev_dve:
                    nc.vector.tensor_scalar_max(
                        khs[bb][:, t0:t0 + tg, :], psK[:, :tg, :], 0.0
                    )
                else:
                    nc.scalar.activation(
                        khs[bb][:, t0:t0 + tg, :], psK[:, :tg, :], AF.Relu
                    )

            def v_group(bb, t0, vev_dve=False):
                g2 = g2s[bb]
                tg = min(4, HIS - t0)
                psV = P1.tile([128, 4, E], f32, tag="p1", name="psV")
                for tt in range(tg):
                    for c in range(2):
                        nc.tensor.matmul(
                            psV[:, tt, :],
                            g2[:, c, t0 + tt, :, :].rearrange("p a b -> p (a b)"),
                            Wkv16[:, c, E:2 * E],
                            start=(c == 0), stop=(c == 1),
                        )
                vout = vh2s[bb][:, t0:t0 + tg, :]
                if vev_dve:
                    nc.vector.tensor_scalar_max(vout, psV[:, :tg, :], 0.0)
                else:
                    nc.scalar.activation(vout, psV[:, :tg, :], AF.Relu)

            def kv_group(bb, t0, kev_dve):
                k_group(bb, t0, kev_dve)
                v_group(bb, t0)

            def att_chunk(bb, h0, split):
                """x = qh*kh for t in [h0, h0+25), d-tree -> att16 slice."""
                kh = khs[bb]
                qh_b = qh16[:, bb, None, :]
                att16 = att16s[bb]
                x = xb.tile([128, 25, NV, DV], f16, tag="x", name="x")
                xf = x.rearrange("p t v d -> p t (v d)")
                if split:  # unused currently: GP on the softmax path is slow
                    nc.vector.tensor_tensor(
                        xf[:, :, 0:JGP], kh[:, h0:h0 + 25, 0:JGP],
                        qh_b[:, :, 0:JGP].to_broadcast((128, 25, JGP)),
                        OP.mult,
                    )
                    nc.gpsimd.tensor_tensor(
                        xf[:, :, JGP:E], kh[:, h0:h0 + 25, JGP:E],
                        qh_b[:, :, JGP:E].to_broadcast((128, 25, E - JGP)),
                        OP.mult,
                    )
                    for w in (16, 8, 4, 2):
                        nc.vector.tensor_tensor(
                            x[:, :, 0:NV - NH_GP, :w],
                            x[:, :, 0:NV - NH_GP, :w],
                            x[:, :, 0:NV - NH_GP, w:2 * w], OP.add,
                        )
                        nc.gpsimd.tensor_tensor(
                            x[:, :, NV - NH_GP:, :w],
                            x[:, :, NV - NH_GP:, :w],
                            x[:, :, NV - NH_GP:, w:2 * w], OP.add,
                        )
                    nc.vector.tensor_tensor(
                        att16[:, 0:NV - NH_GP, h0:h0 + 25].rearrange(
                            "p v t -> p t v"
                        ),
                        x[:, :, 0:NV - NH_GP, 0], x[:, :, 0:NV - NH_GP, 1],
                        OP.add,
                    )
                    nc.gpsimd.tensor_tensor(
                        att16[:, NV - NH_GP:, h0:h0 + 25].rearrange(
                            "p v t -> p t v"
                        ),
                        x[:, :, NV - NH_GP:, 0], x[:, :, NV - NH_GP:, 1],
                        OP.add,
                    )
                else:
                    nc.vector.tensor_tensor(
                        xf, kh[:, h0:h0 + 25, :],
                        qh_b.to_broadcast((128, 25, E)), OP.mult,
                    )
                    for w in (16, 8, 4, 2):
                        nc.vector.tensor_tensor(
                            x[:, :, :, :w], x[:, :, :, :w],
                            x[:, :, :, w:2 * w], OP.add,
                        )
                    nc.vector.tensor_tensor(
                        att16[:, :, h0:h0 + 25].rearrange("p v t -> p t v"),
                        x[:, :, :, 0], x[:, :, :, 1], OP.add,
                    )

            def softmax_pre(bb):
                att16, exp16 = att16s[bb], exp16s[bb]
                nc.scalar.activation(exp16, att16, AF.Exp, scale=1.0 / SQD)
                ssum = small.tile([128, NV], f32, tag="ssum", name="ssum")
                nc.vector.reduce_sum(ssum, exp16, axis=AX.X)
                rinv = small.tile([128, NV], f32, tag="rinv", name="rinv")
                nc.vector.reciprocal(rinv, ssum)
                rinvs[bb] = rinv

            def av_block(bb, nh_gp=0):
                exp16 = exp16s[bb]
                vh = vh2s[bb]
                y_raw = work.tile([128, 2, HIS, 2, 64], f16, tag="g2",
                                  name=f"y{bb}", bufs=2)
                y = y_raw.rearrange("p a t b n -> p (a t b n)").rearrange(
                    "p (t j) -> p t j", j=E
                )
                # y[p,t,(v,d)] = vh * exp16[p,v,t] (GP-broadcast over d).
                # expb reuses the kh slot: att(bb) is done with kh by now.
                expb = work.tile([128, HIS, E], f16, tag="kh",
                                 name=f"expb{bb}", bufs=2)
                expv = expb.rearrange("p t (v d) -> p t v d", v=NV)

                def bsrc(lo, hi):
                    return exp16[:, :, lo:hi].rearrange("p v t -> p t v")[
                        :, :, :, None
                    ].to_broadcast((128, hi - lo, NV, DV))

                # 3-way broadcast split: GP [0,25), ACT [25,39), DVE [39,50)
                nc.gpsimd.tensor_copy(expv[:, 0:25], bsrc(0, 25))
                nc.scalar.copy(expv[:, 25:39], bsrc(25, 39))
                nc.vector.tensor_copy(expv[:, 39:50], bsrc(39, 50))
                for ch in range(2):
                    t0 = 25 * ch
                    nc.vector.tensor_tensor(
                        y[:, t0:t0 + 25, :], vh[:, t0:t0 + 25, :],
                        expb[:, t0:t0 + 25, :], OP.mult,
                    )
                tw = HIS
                while tw > 1:
                    half = tw // 2
                    nc.vector.tensor_tensor(
                        y[:, :half, :], y[:, :half, :],
                        y[:, half:2 * half, :], OP.add,
                    )
                    if tw % 2 == 1:
                        nc.vector.tensor_tensor(
                            y[:, 0, :], y[:, 0, :], y[:, tw - 1, :], OP.add,
                        )
                    tw = half
                avout = small.tile([128, E], f32, tag="avout", name="avout")
                nc.vector.tensor_tensor(
                    avout.rearrange("p (v d) -> p v d", v=NV),
                    y[:, 0, :].rearrange("p (v d) -> p v d", v=NV),
                    rinvs[bb][:, :, None].to_broadcast((128, NV, DV)),
                    OP.mult,
                )
                avouts[bb] = avout

            def out_proj(bb):
                avout = avouts[bb]
                ao_fm = small.tile([128, 2, 128], f16, tag="ao_fm",
                                   name="ao_fm")
                for c in range(2):
                    ps = P2.tile([128, 4, 2, 64], f32, tag="p2", name="ps_ao")
                    psv = ps.rearrange("p a b n -> p (a b n)")
                    nc.tensor.transpose(
                        psv[:, 0:128], avout[:, 128 * c:128 * c + 128], ident
                    )
                    if c == 0:
                        nc.scalar.copy(ao_fm[:, c, :], psv[:, 0:128])
                    else:
                        nc.vector.tensor_copy(ao_fm[:, c, :], psv[:, 0:128])
                pso = P2.tile([128, 4, 2, 64], f32, tag="p2", name="pso")
                psov = pso.rearrange("p a b n -> p (a b n)")
                for c in range(2):
                    nc.tensor.matmul(
                        psov[:, :E], ao_fm[:, c, :], Wo16[:, c, :],
                        start=(c == 0), stop=(c == 1),
                    )
                osb = small.tile([128, E], f32, tag="osb", name="osb")
                nc.scalar.activation(osb, psov[:, :E], AF.Relu)
                nc.sync.dma_start(
                    out_d[2 * bb:2 * bb + 2].rearrange("b n e -> (b n) e"),
                    osb,
                )

            # ============ emission schedule ============
            # enc emitted in pr-chunks of 8 pairs (pc = 0..3); after each
            # pc the pe-add pieces whose gelus completed are emitted, then
            # the kv groups they unlock.
            # pr-chunk 0 covers pairs 0-15 of each parity: t' in
            # [0,16) u [25,41); chunk 1 covers the rest. pe-add pieces must
            # match covered t' ranges exactly.
            PCS = (0, 16)
            PIECES_BY_PC = {
                0: ((0, 16), (25, 41)),
                1: ((16, 25), (41, 50)),
            }

            def enc_pc(bb, pc):
                for q in range(4):
                    for c in range(2):
                        enc_chunk(bb, c, q, PCS[pc])
                for c in range(2):
                    for lo, hi in PIECES_BY_PC[pc]:
                        peadd(bb, c, lo, hi, nc.vector)

            # --- phase B: enc0 + pe-add0 + kv0 groups g0..g9
            KV_B = {0: (0, 4), 1: (4, 10)}
            alloc_g2(0)
            alloc_kv(0)
            for pc in range(2):
                enc_pc(0, pc)
                for g in range(*KV_B[pc]):
                    kv_group(0, 4 * g, kev_dve=True)

            # --- phase C: enc1 + pe-add1, kv0 tail, kv1-K start, att0
            alloc_g2(1)
            alloc_kv(1)
            for bb in range(2):
                att16s[bb] = small.tile([128, NV, HIS], f16, tag="att16",
                                        name=f"att16_{bb}")
                exp16s[bb] = small.tile([128, NV, HIS], f16, tag="exp16",
                                        name=f"exp16_{bb}")
            for pc in range(2):
                enc_pc(1, pc)
                if pc == 0:
                    for g in range(10, 13):
                        kv_group(0, 4 * g, kev_dve=True)
                    att_chunk(0, 0, split=False)
                if pc == 1:
                    att_chunk(0, 25, split=False)
                    for g in range(0, 6):
                        k_group(1, 4 * g, kev_dve=False)

            # --- phase D: kv1-K tail + softmax0/av0/att1-first
            for g in range(6, 13):
                k_group(1, 4 * g, kev_dve=False)
                if g == 7:
                    softmax_pre(0)
                elif g == 9:
                    av_block(0, nh_gp=2)
                elif g == 12:
                    att_chunk(1, 0, split=False)

            # --- phase E2: kv1-V; att1 second half + softmax1 + out0
            for g in range(13):
                v_group(1, 4 * g)
                if g == 0:
                    att_chunk(1, 25, split=False)
                elif g == 2:
                    softmax_pre(1)
                elif g == 4:
                    out_proj(0)

            # --- phase F: tail
            av_block(1, nh_gp=2)
            out_proj(1)

    nc.compile()
    return nc


def build_in_maps(inputs):
    q = np.asarray(inputs["q"], np.float32)
    k = np.asarray(inputs["k"], np.float32)
    adjs = np.asarray(inputs["adjs_pos"], np.float32)
    Wq = np.asarray(inputs["Wq"], np.float32)
    Wk = np.asarray(inputs["Wk"], np.float32)
    Wv = np.asarray(inputs["Wv"], np.float32)
    Wo = np.asarray(inputs["Wout"], np.float32)
    We = np.asarray(inputs["Wenc"], np.float32)

    pe = _positional_encoding()
    # parity-major t axis: t' = tp*NPR + pr maps to t = 2*pr + tp
    tperm = np.concatenate([np.arange(0, HIS, 2), np.arange(1, HIS, 2)])
    pe_pm = pe[tperm]  # [HIS, E] in t'-order
    pe_x2 = np.ascontiguousarray(
        np.broadcast_to(
            pe_pm.T.reshape(2, 128, HIS).transpose(1, 0, 2)[:, :, :, None],
            (128, 2, HIS, 16),
        )
    ).astype(np.float16)

    Wkv = np.concatenate([Wk, Wv], axis=1)
    Wkv16 = np.ascontiguousarray(
        Wkv.reshape(2, 128, 2 * E).transpose(1, 0, 2)
    ).astype(np.float16)
    Wq16 = np.ascontiguousarray(
        Wq.reshape(2, 128, E).transpose(1, 0, 2)
    ).astype(np.float16)
    Wo16 = np.ascontiguousarray(
        Wo.reshape(2, 128, E).transpose(1, 0, 2)
    ).astype(np.float16)

    We_rep4 = np.zeros((128, 2, 128), np.float32)
    for qd in range(4):
        for c in range(2):
            We_rep4[32 * qd:32 * qd + DK, c, :] = We[:, 128 * c:128 * c + 128]
    We_rep4 = We_rep4.astype(np.float16)

    adj2 = np.zeros((128, NPR, 64), np.float32)
    for tp in range(2):
        adj2[64 * tp:64 * tp + 64] = adjs[:, tp::2, :].transpose(2, 1, 0)
    adj2 = adj2.astype(np.float16)

    shared = {
        "adj2": adj2, "Wq16": Wq16, "Wkv16": Wkv16,
        "Wo16": Wo16, "We_rep4": We_rep4, "pe_x2": pe_x2,
    }
    in_maps = []
    for cid in range(NC):
        qc = q[cid * BPC:(cid + 1) * BPC]
        kc = k[cid * BPC:(cid + 1) * BPC]
        q_fm = np.ascontiguousarray(
            qc.reshape(NBB, 2, N, 2, 128).transpose(4, 3, 0, 1, 2).reshape(
                128, 2, NBB, 128
            )
        ).astype(np.float16)
        kg2 = np.zeros((128, NBB, NPR, 128), np.float32)
        for tp in range(2):
            for bbi in range(NBB):
                for b2 in range(2):
                    blk = kc[2 * bbi + b2, tp::2].transpose(1, 0, 2)
                    kg2[64 * tp:64 * tp + 64, bbi, :,
                        64 * tp + 32 * b2:64 * tp + 32 * b2 + DK] = blk
        kg2 = kg2.astype(np.float16)
        m = dict(shared)
        m["q_fm"] = q_fm
        m["kg2"] = kg2
        in_maps.append(m)
    return in_maps


def kernel(**inputs):
    from concourse.bass_utils import run_bass_kernel_spmd

    nc = _build()
    in_maps = build_in_maps(inputs)
    res = run_bass_kernel_spmd(nc, in_maps, core_ids=list(range(NC)))
    return np.concatenate([r["out"] for r in res.results], axis=0)


if __name__ == "__main__":
    import reference

    ins = {k: np.asarray(v) for k, v in reference.setup_inputs().items()}
    out = kernel(**ins)
    expected = np.asarray(reference.reference(**ins))
    err = np.linalg.norm(out - expected) / np.linalg.norm(expected)
    print("out", out.shape, out.dtype, "rel_err", err)
